# revision 11
# baseline (speedup 1.0000x reference)
"""Gaussian-HMM (Kalman) marginal log-likelihood on 8 Trainium2 NeuronCores.

Math (same decomposition as the validated baseline):
  The 64 obs dims split into 4 exchangeable sensor types (16 sensors each).
  60 "static" directions give a closed-form ll from per-sensor sums and
  sums-of-squares; the 4 type-mean series w (T x 4) feed a 2-state LTI
  Kalman filter whose steady-state innovations are an exact FIR of w
  (the filter poles decay at |eig| = 0.03/step, so 2 taps suffice:
  truncation ~1e-6 relative).  E_late = sum_t ||L^T r_t||^2 with
  Sinv_inf = L L^T; folding m4q (sensor->type-mean projection) into the
  whitened FIR gives rho = sum_k A_k^T track^T[:, k:k+510] directly -- no
  intermediate w / im2col needed.  The first 16 global steps use the exact
  time-varying map and the 2 chunk-boundary steps per core the
  steady-state FIR -- both on host from a handful of track rows (O(1)).

Device program per core (10 instructions): 2 parallel input DMAs of the
pre-transposed bf16 chunk (+m4q/A_k columns), bn_stats for per-sensor
sum/ssq, 1 matmul for w and bn_stats of it (sum w^2), 3 PSUM-accumulated
matmuls for rho and bn_stats of it (sum rho, sum rho^2), 1 output DMA.
Sharding: time dimension, 512 steps per core, no halo.
"""
import numpy as np
import ml_dtypes

import concourse.bass as bass
import concourse.mybir as mybir
from concourse import tile
from concourse.bass_utils import run_bass_kernel_spmd

# ---------------------------------------------------------------- constants
T = 4096
LOG2PI = float(np.log(2.0 * np.pi))
NCORES = 8
CHUNK = T // NCORES          # 512
T1 = 16                      # exact-LTV prefix length
LTAP = 1                     # FIR taps (pole magnitude 0.0294; truncation ~1e-5)
TCV = 64                     # steps of exact host recursion (converged long before)
NT = 4 * (LTAP + 1)          # rows of phi
NR = CHUNK - LTAP            # 510 residuals computed on device per core
F32 = mybir.dt.float32
BF16 = mybir.dt.bfloat16
BF16NP = ml_dtypes.bfloat16


def _type_indices():
    # type c = 2*g + p observes state g; sensors i = 32g + 2j + p
    return [np.arange(16) * 2 + (c % 2) + 32 * (c // 2) for c in range(4)]


# ---------------------------------------------------------------- host precompute
def _host_precompute(bias_scales, obs_noise, trans_noise, transition_param):
    """All parameter-dependent matrices/constants, in float64."""
    r = float(obs_noise) ** 2
    q = float(trans_noise[0]) ** 2
    Fs = np.flip(np.diag(transition_param.astype(np.float64)), 0).T
    C = np.zeros((4, 2))
    for c in range(4):
        C[c, c // 2] = 4.0

    P = np.eye(2)
    mc = np.zeros((2, 4))
    Ks, Ss, Ds = [], [], []
    for t in range(TCV):
        mc = Fs @ mc
        P = Fs @ P @ Fs.T + q * np.eye(2)
        Smat = C @ P @ C.T + r * np.eye(4)
        Sinv = np.linalg.inv(Smat)
        D = np.eye(4) - C @ mc
        K = P @ C.T @ Sinv
        mc = mc + K @ D
        P = (np.eye(2) - K @ C) @ P
        P = 0.5 * (P + P.T)
        Ks.append(K); Ss.append(Smat); Ds.append(D)
    S_inf, K_inf, D_inf = Ss[-1], Ks[-1], Ds[-1]
    G_inf = (np.eye(2) - K_inf @ C) @ Fs

    # exact residual map for t < T1 (v = w[0:T1] flattened time-major)
    n = 4 * T1
    Mmat = np.zeros((2, n))
    Atil = np.zeros((n, n))
    Btil = np.zeros((n, 4))
    for t in range(T1):
        E = np.zeros((4, n)); E[:, 4 * t:4 * t + 4] = np.eye(4)
        Row = E - C @ (Fs @ Mmat)
        Li = np.linalg.inv(np.linalg.cholesky(Ss[t]))
        Atil[4 * t:4 * t + 4] = Li @ Row
        Btil[4 * t:4 * t + 4] = Li @ Ds[t]
        Mmat = Fs @ Mmat + Ks[t] @ Row

    taps = np.zeros((LTAP, 4, 4))
    Gk = np.eye(2)
    for k in range(LTAP):
        taps[k] = C @ Fs @ Gk @ K_inf
        Gk = G_inf @ Gk

    sum_logdet = sum(np.linalg.slogdet(Sm)[1] for Sm in Ss) \
        + (T - TCV) * np.linalg.slogdet(S_inf)[1]
    Lam = sum(D.T @ np.linalg.inv(Sm) @ D for D, Sm in zip(Ds, Ss)) \
        + (T - TCV) * (D_inf.T @ np.linalg.inv(S_inf) @ D_inf)

    Sinv_inf = np.linalg.inv(S_inf)
    L = np.linalg.cholesky(Sinv_inf)              # L @ L.T = Sinv_inf

    # whitened FIR: rho_t = phi^T x_t, x_t[(LTAP+1)c + k] = w[c, t-LTAP+k]
    psi = np.zeros((NT, 4))
    for c in range(4):
        for k in range(LTAP + 1):
            p = (LTAP + 1) * c + k
            if k == LTAP:
                psi[p, c] = 1.0
            else:
                psi[p, :] = -taps[LTAP - 1 - k][:, c]
    phi = psi @ L

    idx = _type_indices()
    m4q = np.zeros((64, 4))
    for c, ids in enumerate(idx):
        m4q[ids, c] = 0.25
    # fold sensor->type projection into the FIR: rho[:,tau] = sum_k A_k^T y_{tau+k}
    A = np.zeros((LTAP + 1, 64, 4))
    for k in range(LTAP + 1):
        Phik = np.stack([phi[(LTAP + 1) * c + k] for c in range(4)])
        A[k] = m4q @ Phik
    return dict(r=r, q=q, Fs=Fs, Atil=Atil, Btil=Btil, sum_logdet=sum_logdet,
                Lam=Lam, S_inf=S_inf, Sinv_inf=Sinv_inf, D_inf=D_inf, L=L,
                phi=phi, m4q=m4q, A=A,
                bias_scales=np.asarray(bias_scales, np.float64))


# ---------------------------------------------------------------- bass kernel
def _split_multi_waits(nc):
    """This container's walrus rejects >1 sem wait per instruction: peel the
    extras onto engine-tagged NoOp carriers inserted just before."""
    cnt = 0
    for fn in nc.m.functions:
        for blk in fn.blocks:
            out = []
            changed = False
            for inst in blk.instructions:
                si = getattr(inst, "sync_info", None)
                waits = list(si.on_wait) if si is not None else []
                if len(waits) > 1:
                    changed = True
                    for w in waits[:-1]:
                        cnt += 1
                        nop = mybir.InstNoOp(name=f"I-wsplit-{cnt}", ins=[], outs=[])
                        nop.engine = inst.engine
                        nop.sync_info = mybir.SyncInfo(on_wait=[w], on_update=[])
                        out.append(nop)
                    inst.sync_info = mybir.SyncInfo(
                        on_wait=[waits[-1]], on_update=list(si.on_update)
                    )
                out.append(inst)
            if changed:
                blk.instructions = out
    return cnt


_NC_CACHE = {}

# TK columns: 0:512 transposed track chunk, 512+8k:520+8k = [A_k | m4q*(k==LTAP)]
TKW = 512 + 8 * (LTAP + 1)   # 528


def _build_nc():
    if "nc" in _NC_CACHE:
        return _NC_CACHE["nc"]

    nc = bass.Bass("TRN2", target_bir_lowering=False, debug=False,
                   num_devices=NCORES)
    tk_d = nc.declare_dram_parameter("tk", [64, TKW], BF16, isOutput=False)
    o_pack = nc.declare_dram_parameter("o_pack", [64, 12], F32, isOutput=True)

    with tile.TileContext(nc) as tc:
        with (
            tc.tile_pool(name="sb", bufs=1) as sb,
            tc.tile_pool(name="ps", bufs=1, space="PSUM") as ps,
        ):
            TK = sb.tile([64, TKW], BF16)
            nc.sync.dma_start(TK[:], tk_d[:])
            pack = sb.tile([64, 12], F32)
            nc.gpsimd.memset(pack[:], 0.0)

            # per-sensor sum / sum-of-squares stats
            nc.vector.bn_stats(pack[0:64, 0:6], TK[:, 0:512])

            # fused [rho | w] = sum_k [A_k | m4q*(k==LTAP)]^T y_{tau+k}:
            # rows 0:4 whitened residuals (steps tau+LTAP), rows 4:8 type
            # means w at steps tau+LTAP (chunk-local steps LTAP..511)
            rw = ps.tile([8, NR], F32)
            for k in range(LTAP + 1):
                nc.tensor.matmul(rw[:], TK[:, 512 + 8 * k:520 + 8 * k],
                                 TK[:, k:k + NR],
                                 start=(k == 0), stop=(k == LTAP))
            nc.vector.bn_stats(pack[0:8, 6:12], rw[:])

            nc.scalar.dma_start(o_pack[:], pack[:])

    _split_multi_waits(nc)
    _NC_CACHE["nc"] = nc
    return nc


# ---------------------------------------------------------------- host assembly
def _bn_sums(p6):
    """(sum, sum-of-squares) per partition from bn_stats 6-tuple columns."""
    ce, me, cve = p6[..., 0], p6[..., 1], p6[..., 2]
    co, mo, cvo = p6[..., 3], p6[..., 4], p6[..., 5]
    return ce * me + co * mo, cve + ce * me ** 2 + cvo + co * mo ** 2


def _assemble(pre, g, ssq_s, sw2, sumrho, ssqrho, track):
    """Combine device stats into the final log-likelihood (float64)."""
    r = pre["r"]
    bs = pre["bias_scales"]
    idx = _type_indices()
    phi = pre["phi"]
    m4q = pre["m4q"]
    ll = 0.0
    # device w-stats cover chunk-local steps LTAP..511: add the missing ones
    sw2 = sw2.copy()
    for j in range(NCORES):
        wm = (track[CHUNK * j:CHUNK * j + LTAP] @ m4q).T
        sw2 += (wm ** 2).sum(axis=1)
    # static directions: 15 per type
    for c, ids in enumerate(idx):
        v = bs[c % 2]
        ssq = ssq_s[ids].sum()
        tp2 = 16.0 * sw2[c]
        Gc = g[ids]
        ssq_rest = ssq - tp2 / 16.0
        g_rest = (Gc ** 2).sum() - (Gc.sum() ** 2) / 16.0
        quad = (ssq_rest - (v / (r + T * v)) * g_rest) / r
        ll += -0.5 * quad - 0.5 * 15 * ((T - 1) * np.log(r) + np.log(r + T * v)) \
              - 0.5 * 15 * T * LOG2PI

    # device rho covers steps [512j+LTAP, 512(j+1)).  Add the steady-state
    # FIR for boundary steps [512j, 512j+LTAP) of cores j>=1; swap core 0's
    # steady-state steps [LTAP, 16) for the exact map on [0, 16).
    E_late = ssqrho.sum()
    srho = sumrho.copy()
    for j in range(1, NCORES):
        w8 = (track[CHUNK * j - LTAP:CHUNK * j + 2 * LTAP] @ m4q).T
        for i in range(LTAP):
            rho_t = phi.T @ w8[:, i:i + LTAP + 1].reshape(-1)
            E_late += rho_t @ rho_t
            srho += rho_t
    w16 = (track[0:T1] @ m4q).T                      # [4, 16]
    for t in range(LTAP, T1):
        rho_t = phi.T @ w16[:, t - LTAP:t + 1].reshape(-1)
        E_late -= rho_t @ rho_t
        srho -= rho_t
    rl = np.linalg.solve(pre["L"].T, srho)

    v_flat = w16.T.reshape(-1)                       # v[4t+c] = w16[c, t]
    re = pre["Atil"] @ v_flat
    E_early = float(re @ re)
    b_early = pre["Btil"].T @ re

    Sinv_inf = pre["Sinv_inf"]
    b = b_early + pre["D_inf"].T @ Sinv_inf @ rl
    ll += -0.5 * (E_early + E_late) - 0.5 * pre["sum_logdet"] - 0.5 * 4 * T * LOG2PI
    Sb = np.diag([bs[c % 2] for c in range(4)])
    ll += -0.5 * np.linalg.slogdet(np.eye(4) + Sb @ pre["Lam"])[1]
    ll += 0.5 * b @ np.linalg.solve(np.linalg.inv(Sb) + pre["Lam"], b)
    return ll


def _make_in_maps(track, pre):
    track = np.ascontiguousarray(track, np.float32)
    in_maps = []
    for j in range(NCORES):
        tk = np.zeros((64, TKW), BF16NP)
        tk[:, 0:512] = track[CHUNK * j:CHUNK * (j + 1)].T
        for k in range(LTAP + 1):
            tk[:, 512 + 8 * k:516 + 8 * k] = pre["A"][k]
        tk[:, 516 + 8 * LTAP:520 + 8 * LTAP] = pre["m4q"]
        in_maps.append({"tk": tk})
    return in_maps


def kernel(track, bias_scales, obs_noise, trans_noise, transition_param,
           _trace=False):
    track = np.asarray(track)
    pre = _host_precompute(np.asarray(bias_scales), np.asarray(obs_noise),
                           np.asarray(trans_noise), np.asarray(transition_param))
    nc = _build_nc()
    in_maps = _make_in_maps(track, pre)
    res = run_bass_kernel_spmd(nc, in_maps, list(range(NCORES)), trace=_trace)
    g = np.zeros(64, np.float64)
    ssq_s = np.zeros(64, np.float64)
    sw2 = np.zeros(4, np.float64)
    sumrho = np.zeros(4, np.float64)
    ssqrho = np.zeros(4, np.float64)
    for j in range(NCORES):
        p = res.results[j]["o_pack"].astype(np.float64)
        s, ss = _bn_sums(p[0:64, 0:6])
        g += s; ssq_s += ss
        s, ss = _bn_sums(p[0:8, 6:12])
        sumrho += s[0:4]; ssqrho += ss[0:4]
        sw2 += ss[4:8]
    ll = _assemble(pre, g, ssq_s, sw2, sumrho, ssqrho,
                   np.asarray(track, np.float64))
    if _trace:
        kernel._last_exec_time_ns = res.exec_time_ns
    return np.float32(ll)


# revision 12
# speedup vs baseline: 1.0268x; 1.0268x over previous
"""Gaussian-HMM (Kalman) marginal log-likelihood on 8 Trainium2 NeuronCores.

Math (same decomposition as the validated baseline):
  The 64 obs dims split into 4 exchangeable sensor types (16 sensors each).
  60 "static" directions give a closed-form ll from per-sensor sums and
  sums-of-squares; the 4 type-mean series w (T x 4) feed a 2-state LTI
  Kalman filter whose steady-state innovations are an exact FIR of w
  (the filter poles decay at |eig| = 0.03/step, so 2 taps suffice:
  truncation ~1e-6 relative).  E_late = sum_t ||L^T r_t||^2 with
  Sinv_inf = L L^T; folding m4q (sensor->type-mean projection) into the
  whitened FIR gives rho = sum_k A_k^T track^T[:, k:k+510] directly -- no
  intermediate w / im2col needed.  The first 16 global steps use the exact
  time-varying map and the 2 chunk-boundary steps per core the
  steady-state FIR -- both on host from a handful of track rows (O(1)).

Device program per core (10 instructions): 2 parallel input DMAs of the
pre-transposed bf16 chunk (+m4q/A_k columns), bn_stats for per-sensor
sum/ssq, 1 matmul for w and bn_stats of it (sum w^2), 3 PSUM-accumulated
matmuls for rho and bn_stats of it (sum rho, sum rho^2), 1 output DMA.
Sharding: time dimension, 512 steps per core, no halo.
"""
import numpy as np
import ml_dtypes

import concourse.bass as bass
import concourse.mybir as mybir
from concourse import tile
from concourse.bass_utils import run_bass_kernel_spmd

# ---------------------------------------------------------------- constants
T = 4096
LOG2PI = float(np.log(2.0 * np.pi))
NCORES = 8
CHUNK = T // NCORES          # 512
T1 = 16                      # exact-LTV prefix length
LTAP = 1                     # FIR taps (pole magnitude 0.0294; truncation ~1e-5)
TCV = 64                     # steps of exact host recursion (converged long before)
NT = 4 * (LTAP + 1)          # rows of phi
NR = CHUNK - LTAP            # 510 residuals computed on device per core
F32 = mybir.dt.float32
BF16 = mybir.dt.bfloat16
BF16NP = ml_dtypes.bfloat16


def _type_indices():
    # type c = 2*g + p observes state g; sensors i = 32g + 2j + p
    return [np.arange(16) * 2 + (c % 2) + 32 * (c // 2) for c in range(4)]


# ---------------------------------------------------------------- host precompute
def _host_precompute(bias_scales, obs_noise, trans_noise, transition_param):
    """All parameter-dependent matrices/constants, in float64."""
    r = float(obs_noise) ** 2
    q = float(trans_noise[0]) ** 2
    Fs = np.flip(np.diag(transition_param.astype(np.float64)), 0).T
    C = np.zeros((4, 2))
    for c in range(4):
        C[c, c // 2] = 4.0

    P = np.eye(2)
    mc = np.zeros((2, 4))
    Ks, Ss, Ds = [], [], []
    for t in range(TCV):
        mc = Fs @ mc
        P = Fs @ P @ Fs.T + q * np.eye(2)
        Smat = C @ P @ C.T + r * np.eye(4)
        Sinv = np.linalg.inv(Smat)
        D = np.eye(4) - C @ mc
        K = P @ C.T @ Sinv
        mc = mc + K @ D
        P = (np.eye(2) - K @ C) @ P
        P = 0.5 * (P + P.T)
        Ks.append(K); Ss.append(Smat); Ds.append(D)
    S_inf, K_inf, D_inf = Ss[-1], Ks[-1], Ds[-1]
    G_inf = (np.eye(2) - K_inf @ C) @ Fs

    # exact residual map for t < T1 (v = w[0:T1] flattened time-major)
    n = 4 * T1
    Mmat = np.zeros((2, n))
    Atil = np.zeros((n, n))
    Btil = np.zeros((n, 4))
    for t in range(T1):
        E = np.zeros((4, n)); E[:, 4 * t:4 * t + 4] = np.eye(4)
        Row = E - C @ (Fs @ Mmat)
        Li = np.linalg.inv(np.linalg.cholesky(Ss[t]))
        Atil[4 * t:4 * t + 4] = Li @ Row
        Btil[4 * t:4 * t + 4] = Li @ Ds[t]
        Mmat = Fs @ Mmat + Ks[t] @ Row

    taps = np.zeros((LTAP, 4, 4))
    Gk = np.eye(2)
    for k in range(LTAP):
        taps[k] = C @ Fs @ Gk @ K_inf
        Gk = G_inf @ Gk

    sum_logdet = sum(np.linalg.slogdet(Sm)[1] for Sm in Ss) \
        + (T - TCV) * np.linalg.slogdet(S_inf)[1]
    Lam = sum(D.T @ np.linalg.inv(Sm) @ D for D, Sm in zip(Ds, Ss)) \
        + (T - TCV) * (D_inf.T @ np.linalg.inv(S_inf) @ D_inf)

    Sinv_inf = np.linalg.inv(S_inf)
    L = np.linalg.cholesky(Sinv_inf)              # L @ L.T = Sinv_inf

    # whitened FIR: rho_t = phi^T x_t, x_t[(LTAP+1)c + k] = w[c, t-LTAP+k]
    psi = np.zeros((NT, 4))
    for c in range(4):
        for k in range(LTAP + 1):
            p = (LTAP + 1) * c + k
            if k == LTAP:
                psi[p, c] = 1.0
            else:
                psi[p, :] = -taps[LTAP - 1 - k][:, c]
    phi = psi @ L

    idx = _type_indices()
    m4q = np.zeros((64, 4))
    for c, ids in enumerate(idx):
        m4q[ids, c] = 0.25
    # fold sensor->type projection into the FIR: rho[:,tau] = sum_k A_k^T y_{tau+k}
    A = np.zeros((LTAP + 1, 64, 4))
    for k in range(LTAP + 1):
        Phik = np.stack([phi[(LTAP + 1) * c + k] for c in range(4)])
        A[k] = m4q @ Phik
    return dict(r=r, q=q, Fs=Fs, Atil=Atil, Btil=Btil, sum_logdet=sum_logdet,
                Lam=Lam, S_inf=S_inf, Sinv_inf=Sinv_inf, D_inf=D_inf, L=L,
                phi=phi, m4q=m4q, A=A,
                bias_scales=np.asarray(bias_scales, np.float64))


# ---------------------------------------------------------------- bass kernel
def _split_multi_waits(nc):
    """This container's walrus rejects >1 sem wait per instruction: peel the
    extras onto engine-tagged NoOp carriers inserted just before."""
    cnt = 0
    for fn in nc.m.functions:
        for blk in fn.blocks:
            out = []
            changed = False
            for inst in blk.instructions:
                si = getattr(inst, "sync_info", None)
                waits = list(si.on_wait) if si is not None else []
                if len(waits) > 1:
                    changed = True
                    for w in waits[:-1]:
                        cnt += 1
                        nop = mybir.InstNoOp(name=f"I-wsplit-{cnt}", ins=[], outs=[])
                        nop.engine = inst.engine
                        nop.sync_info = mybir.SyncInfo(on_wait=[w], on_update=[])
                        out.append(nop)
                    inst.sync_info = mybir.SyncInfo(
                        on_wait=[waits[-1]], on_update=list(si.on_update)
                    )
                out.append(inst)
            if changed:
                blk.instructions = out
    return cnt


_NC_CACHE = {}

# TK columns: 0:512 transposed track chunk, 512+8k:520+8k = [A_k | m4q*(k==LTAP)]
TKW = 512 + 8 * (LTAP + 1)   # 528


def _build_nc():
    if "nc" in _NC_CACHE:
        return _NC_CACHE["nc"]

    nc = bass.Bass("TRN2", target_bir_lowering=False, debug=False,
                   num_devices=NCORES)
    tk_d = nc.declare_dram_parameter("tk", [64, TKW], BF16, isOutput=False)
    o_pack = nc.declare_dram_parameter("o_pack", [64, 12], F32, isOutput=True)

    with tile.TileContext(nc) as tc:
        with (
            tc.tile_pool(name="sb", bufs=1) as sb,
            tc.tile_pool(name="ps", bufs=1, space="PSUM") as ps,
        ):
            TK = sb.tile([64, TKW], BF16)
            nc.sync.dma_start(TK[:], tk_d[:], single_packet=True)
            pack = sb.tile([64, 12], F32)
            nc.gpsimd.memset(pack[:], 0.0)

            # per-sensor sum / sum-of-squares stats
            nc.vector.bn_stats(pack[0:64, 0:6], TK[:, 0:512])

            # fused [rho | w] = sum_k [A_k | m4q*(k==LTAP)]^T y_{tau+k}:
            # rows 0:4 whitened residuals (steps tau+LTAP), rows 4:8 type
            # means w at steps tau+LTAP (chunk-local steps LTAP..511)
            rw = ps.tile([8, NR], F32)
            for k in range(LTAP + 1):
                nc.tensor.matmul(rw[:], TK[:, 512 + 8 * k:520 + 8 * k],
                                 TK[:, k:k + NR],
                                 start=(k == 0), stop=(k == LTAP))
            nc.vector.bn_stats(pack[0:8, 6:12], rw[:])

            nc.scalar.dma_start(o_pack[:], pack[:], single_packet=True)

    _split_multi_waits(nc)
    _NC_CACHE["nc"] = nc
    return nc


# ---------------------------------------------------------------- host assembly
def _bn_sums(p6):
    """(sum, sum-of-squares) per partition from bn_stats 6-tuple columns."""
    ce, me, cve = p6[..., 0], p6[..., 1], p6[..., 2]
    co, mo, cvo = p6[..., 3], p6[..., 4], p6[..., 5]
    return ce * me + co * mo, cve + ce * me ** 2 + cvo + co * mo ** 2


def _assemble(pre, g, ssq_s, sw2, sumrho, ssqrho, track):
    """Combine device stats into the final log-likelihood (float64)."""
    r = pre["r"]
    bs = pre["bias_scales"]
    idx = _type_indices()
    phi = pre["phi"]
    m4q = pre["m4q"]
    ll = 0.0
    # device w-stats cover chunk-local steps LTAP..511: add the missing ones
    sw2 = sw2.copy()
    for j in range(NCORES):
        wm = (track[CHUNK * j:CHUNK * j + LTAP] @ m4q).T
        sw2 += (wm ** 2).sum(axis=1)
    # static directions: 15 per type
    for c, ids in enumerate(idx):
        v = bs[c % 2]
        ssq = ssq_s[ids].sum()
        tp2 = 16.0 * sw2[c]
        Gc = g[ids]
        ssq_rest = ssq - tp2 / 16.0
        g_rest = (Gc ** 2).sum() - (Gc.sum() ** 2) / 16.0
        quad = (ssq_rest - (v / (r + T * v)) * g_rest) / r
        ll += -0.5 * quad - 0.5 * 15 * ((T - 1) * np.log(r) + np.log(r + T * v)) \
              - 0.5 * 15 * T * LOG2PI

    # device rho covers steps [512j+LTAP, 512(j+1)).  Add the steady-state
    # FIR for boundary steps [512j, 512j+LTAP) of cores j>=1; swap core 0's
    # steady-state steps [LTAP, 16) for the exact map on [0, 16).
    E_late = ssqrho.sum()
    srho = sumrho.copy()
    for j in range(1, NCORES):
        w8 = (track[CHUNK * j - LTAP:CHUNK * j + 2 * LTAP] @ m4q).T
        for i in range(LTAP):
            rho_t = phi.T @ w8[:, i:i + LTAP + 1].reshape(-1)
            E_late += rho_t @ rho_t
            srho += rho_t
    w16 = (track[0:T1] @ m4q).T                      # [4, 16]
    for t in range(LTAP, T1):
        rho_t = phi.T @ w16[:, t - LTAP:t + 1].reshape(-1)
        E_late -= rho_t @ rho_t
        srho -= rho_t
    rl = np.linalg.solve(pre["L"].T, srho)

    v_flat = w16.T.reshape(-1)                       # v[4t+c] = w16[c, t]
    re = pre["Atil"] @ v_flat
    E_early = float(re @ re)
    b_early = pre["Btil"].T @ re

    Sinv_inf = pre["Sinv_inf"]
    b = b_early + pre["D_inf"].T @ Sinv_inf @ rl
    ll += -0.5 * (E_early + E_late) - 0.5 * pre["sum_logdet"] - 0.5 * 4 * T * LOG2PI
    Sb = np.diag([bs[c % 2] for c in range(4)])
    ll += -0.5 * np.linalg.slogdet(np.eye(4) + Sb @ pre["Lam"])[1]
    ll += 0.5 * b @ np.linalg.solve(np.linalg.inv(Sb) + pre["Lam"], b)
    return ll


def _make_in_maps(track, pre):
    track = np.ascontiguousarray(track, np.float32)
    in_maps = []
    for j in range(NCORES):
        tk = np.zeros((64, TKW), BF16NP)
        tk[:, 0:512] = track[CHUNK * j:CHUNK * (j + 1)].T
        for k in range(LTAP + 1):
            tk[:, 512 + 8 * k:516 + 8 * k] = pre["A"][k]
        tk[:, 516 + 8 * LTAP:520 + 8 * LTAP] = pre["m4q"]
        in_maps.append({"tk": tk})
    return in_maps


def kernel(track, bias_scales, obs_noise, trans_noise, transition_param,
           _trace=False):
    track = np.asarray(track)
    pre = _host_precompute(np.asarray(bias_scales), np.asarray(obs_noise),
                           np.asarray(trans_noise), np.asarray(transition_param))
    nc = _build_nc()
    in_maps = _make_in_maps(track, pre)
    res = run_bass_kernel_spmd(nc, in_maps, list(range(NCORES)), trace=_trace)
    g = np.zeros(64, np.float64)
    ssq_s = np.zeros(64, np.float64)
    sw2 = np.zeros(4, np.float64)
    sumrho = np.zeros(4, np.float64)
    ssqrho = np.zeros(4, np.float64)
    for j in range(NCORES):
        p = res.results[j]["o_pack"].astype(np.float64)
        s, ss = _bn_sums(p[0:64, 0:6])
        g += s; ssq_s += ss
        s, ss = _bn_sums(p[0:8, 6:12])
        sumrho += s[0:4]; ssqrho += ss[0:4]
        sw2 += ss[4:8]
    ll = _assemble(pre, g, ssq_s, sw2, sumrho, ssqrho,
                   np.asarray(track, np.float64))
    if _trace:
        kernel._last_exec_time_ns = res.exec_time_ns
    return np.float32(ll)
